# revision 1
# baseline (speedup 1.0000x reference)
"""Trainium2 Bass kernel for nn_AnchorFreeHead (ragged segment mean-pool +
residual MLP + L2-normalize + contrastive CE loss).

Sharding: data-parallel over the batch (video) dim B=8 — one batch per
NeuronCore. FeatureProj weights and text_feat are replicated. Each core
computes the partial loss sum over its P=128 segments; the 8 partial sums are
averaged on the host (equivalent to the all-reduce of the scalar loss).

Algorithm (per core, batch b):
  The reference's cumsum-then-gather segment mean-pool is reformulated as a
  dense masked matmul: seg_sum[p, d] = sum_t ind[t, p] * img[t, d], with the
  0/1 indicator ind[t, p] = (seg_start[p] <= t < seg_start[p]+seg_len[p])
  generated on-chip from an iota + two compares (exact small-integer fp32
  arithmetic — no data-dependent addressing). The text-embedding gather is
  replaced by similarities against ALL C=200 text rows + one-hot reductions.

  Only 128-row tiles that intersect at least one segment are shipped to the
  device: the host compacts each core's image rows to its covered tiles
  (padded to a common count so all 8 cores run the same SPMD graph) and
  passes the per-tile row offsets as data; padding tiles contribute zero
  to every segment sum by construction.
"""

import dataclasses
import numpy as np
from contextlib import ExitStack

import concourse.bass as bass
import concourse.tile as tile
from concourse import bacc, masks, mybir
from concourse.bass_utils import run_bass_kernel_spmd
import concourse.bass_utils as _bu

# Enable walrus weight-load double-buffering (LDWEIGHTS/MATMUL overlap): the
# default pipeline passes --enable-ldw-opt=false, which leaves every 128-col
# weight load exposed (~267 ns x ~140 matmuls here). Correctness is verified
# against the reference output.
_orig_run_command = _bu.run_command


def _patched_run_command(cmd, **kw):
    if isinstance(cmd, list):
        cmd = ["--enable-ldw-opt=true" if c == "--enable-ldw-opt=false" else c
               for c in cmd]
    return _orig_run_command(cmd, **kw)


_bu.run_command = _patched_run_command

F32 = mybir.dt.float32
F32R = mybir.dt.float32r
I32 = mybir.dt.int32
OP = mybir.AluOpType
ACT = mybir.ActivationFunctionType

B, T, D, P, C, NEG, H = 8, 20000, 512, 128, 200, 3, 256
TT = 128          # rows per matmul tile (contraction chunk)
GT = 4            # tiles per DMA group (1 MiB per dma_start)
PAD_SHIFT = 1.0e6  # negated-shift value that forces ind == 0 for padding tiles


def covered_tiles(seg_start, seg_len, t_len):
    """Indices of TT-row tiles that intersect at least one segment."""
    nt = (t_len + TT - 1) // TT
    cov = np.zeros(nt, bool)
    for s, l in zip(np.asarray(seg_start), np.asarray(seg_len)):
        cov[s // TT:(s + l - 1) // TT + 1] = True
    return np.flatnonzero(cov)


def build_kernel(nt_pad, img_bufs: int = 6):
    """Build the per-core Bass program for nt_pad compacted 128-row tiles."""
    nc = bacc.Bacc("TRN2", target_bir_lowering=False, debug=False, num_devices=8)

    img = nc.dram_tensor("img", [nt_pad * TT, D], F32R, kind="ExternalInput")
    shifts = nc.dram_tensor("shifts", [1, nt_pad], F32, kind="ExternalInput")
    text = nc.dram_tensor("text", [C, D], F32, kind="ExternalInput")
    w1 = nc.dram_tensor("w1", [D, H], F32, kind="ExternalInput")
    w2 = nc.dram_tensor("w2", [H, D], F32, kind="ExternalInput")
    b1 = nc.dram_tensor("b1", [1, H], F32, kind="ExternalInput")
    b2 = nc.dram_tensor("b2", [1, D], F32, kind="ExternalInput")
    seg_start = nc.dram_tensor("seg_start", [1, P], I32, kind="ExternalInput")
    seg_len = nc.dram_tensor("seg_len", [1, P], I32, kind="ExternalInput")
    labels = nc.dram_tensor("labels", [1, P], I32, kind="ExternalInput")
    neg = nc.dram_tensor("neg_idx", [P, NEG], I32, kind="ExternalInput")
    out = nc.dram_tensor("out", [1, 1], F32, kind="ExternalOutput")

    assert nt_pad % GT == 0
    n_groups = nt_pad // GT

    with tile.TileContext(nc) as tc, ExitStack() as ctx:
        con = ctx.enter_context(tc.tile_pool(name="con", bufs=1))
        ep = ctx.enter_context(tc.tile_pool(name="ep", bufs=1))
        img_pool = ctx.enter_context(tc.tile_pool(name="img", bufs=img_bufs))
        ind_pool = ctx.enter_context(tc.tile_pool(name="ind", bufs=3))
        ps_seg = ctx.enter_context(tc.tile_pool(name="ps_seg", bufs=1, space="PSUM"))
        ps_wk = ctx.enter_context(tc.tile_pool(name="ps_wk", bufs=2, space="PSUM"))
        ps_mlp = ctx.enter_context(tc.tile_pool(name="ps_mlp", bufs=1, space="PSUM"))

        # ---- critical-path setup: cf1/cf2 + per-tile shift columns ----------
        ss_row = con.tile([1, P], I32)
        nc.sync.dma_start(ss_row[:], seg_start[:])
        sl_row = con.tile([1, P], I32)
        nc.sync.dma_start(sl_row[:], seg_len[:])
        sh_row = con.tile([1, nt_pad], F32)
        nc.sync.dma_start(sh_row[:], shifts[:])
        ss_row_f = con.tile([1, P], F32)
        nc.vector.tensor_copy(ss_row_f[:], ss_row[:])
        sl_row_f = con.tile([1, P], F32)
        nc.vector.tensor_copy(sl_row_f[:], sl_row[:])

        ones_row = con.tile([1, 128], F32)
        nc.gpsimd.memset(ones_row[:], 1.0)
        # broadcast rows across partitions via rank-1 matmuls (PE is idle here)
        ss_b_ps = ps_wk.tile([128, P], F32, tag="ps_wk")
        nc.tensor.matmul(ss_b_ps[:], ones_row[:], ss_row_f[:], start=True, stop=True)
        ss_b = con.tile([128, P], F32)
        nc.vector.tensor_copy(ss_b[:], ss_b_ps[:])
        sl_b_ps = ps_wk.tile([128, P], F32, tag="ps_wk")
        nc.tensor.matmul(sl_b_ps[:], ones_row[:], sl_row_f[:], start=True, stop=True)
        sl_b = con.tile([128, P], F32)
        nc.vector.tensor_copy(sl_b[:], sl_b_ps[:])
        sh_b_ps = ps_wk.tile([128, nt_pad], F32, tag="ps_wk")
        nc.tensor.matmul(sh_b_ps[:], ones_row[:], sh_row[:], start=True, stop=True)
        sh_b = con.tile([128, nt_pad], F32)
        nc.vector.tensor_copy(sh_b[:], sh_b_ps[:])

        # iota over partitions: val[t, q] = t
        iota_p = con.tile([128, P], I32)
        nc.gpsimd.iota(iota_p[:], pattern=[[0, P]], base=0, channel_multiplier=1)
        iota_f = con.tile([128, P], F32)
        nc.vector.tensor_copy(iota_f[:], iota_p[:])
        # cf1[t, p] = t - seg_start[p];  cf2[t, p] = cf1 - seg_len[p]
        # tile k (original tile o_k, negated shift ns_k = -TT*o_k, as data):
        #   x = t_global - start = cf1 - ns_k
        #   ind = (x >= 0) - (x >= len) = (cf1 >= ns_k) - (cf2 >= ns_k)
        cf1 = con.tile([128, P], F32)
        nc.vector.tensor_tensor(cf1[:], iota_f[:], ss_b[:], op=OP.subtract)
        cf2 = con.tile([128, P], F32)
        nc.vector.tensor_tensor(cf2[:], cf1[:], sl_b[:], op=OP.subtract)

        # broadcast views: [t, j, p] with j (tile-in-group) broadcast for cf*,
        # p broadcast for the per-tile shift columns
        def _bc_cf(ap):
            return dataclasses.replace(ap, ap=[ap.ap[0], [0, GT], ap.ap[1]])

        cf1_bc = _bc_cf(cf1[:])
        cf2_bc = _bc_cf(cf2[:])

        # ---- indicators for ALL tiles up front (depend only on setup data) --
        ind_all = con.tile([128, nt_pad, P], F32R)
        for g in range(n_groups):
            nsh = sh_b[:, g * GT:(g + 1) * GT]
            nsh_bc = dataclasses.replace(nsh, ap=[nsh.ap[0], nsh.ap[1], [0, P]])
            g1 = ind_pool.tile([128, GT, P], F32, tag="g1")
            nc.vector.tensor_tensor(g1[:], cf1_bc, nsh_bc, op=OP.is_ge)
            g2 = ind_pool.tile([128, GT, P], F32, tag="g2")
            nc.vector.tensor_tensor(g2[:], cf2_bc, nsh_bc, op=OP.is_ge)
            nc.vector.tensor_tensor(ind_all[:, g * GT:(g + 1) * GT, :],
                                    g1[:], g2[:], op=OP.subtract)

        # ---- main loop: seg_sum[p, d] via indicator matmuls -----------------
        psum_seg = ps_seg.tile([128, D], F32)
        for g in range(n_groups):
            grp = img_pool.tile([128, GT, D], F32R, tag="img")
            nc.sync.dma_start(
                grp[:], img[g * GT * TT:(g + 1) * GT * TT, :]
                .rearrange("(j p) d -> p j d", p=128))
            for j in range(GT):
                k = g * GT + j
                nc.tensor.matmul(psum_seg[:], ind_all[:, k, :], grp[:, j, :],
                                 start=(k == 0), stop=(k == nt_pad - 1))

        # ---- deferred setup (only needed by the epilogue) -------------------
        identity = con.tile([128, 128], F32)
        masks.make_identity(nc, identity[:])

        sl_col_i = con.tile([P, 1], I32)
        nc.sync.dma_start(sl_col_i[:], seg_len.ap().rearrange("o p -> p o"))
        sl_col_f = con.tile([P, 1], F32)
        nc.vector.tensor_copy(sl_col_f[:], sl_col_i[:])
        recip_len = con.tile([P, 1], F32)
        nc.vector.reciprocal(recip_len[:], sl_col_f[:])

        lab_col_i = con.tile([P, 1], I32)
        nc.sync.dma_start(lab_col_i[:], labels.ap().rearrange("o p -> p o"))
        lab_col = con.tile([P, 1], F32)
        nc.vector.tensor_copy(lab_col[:], lab_col_i[:])
        neg_col_i = con.tile([P, NEG], I32)
        nc.sync.dma_start(neg_col_i[:], neg.ap())
        neg_col = con.tile([P, NEG], F32)
        nc.vector.tensor_copy(neg_col[:], neg_col_i[:])

        iota_c = con.tile([128, C], I32)
        nc.gpsimd.iota(iota_c[:], pattern=[[1, C]], base=0, channel_multiplier=0)
        iota_c_f = con.tile([128, C], F32)
        nc.vector.tensor_copy(iota_c_f[:], iota_c[:])

        w1_sb = con.tile([128, D // 128, H], F32)
        nc.sync.dma_start(w1_sb[:], w1.ap().rearrange("(c k) h -> k c h", k=128))
        w2_sb = con.tile([128, H // 128, D], F32)
        nc.sync.dma_start(w2_sb[:], w2.ap().rearrange("(c k) d -> k c d", k=128))
        b1_sb = con.tile([1, H], F32)
        nc.sync.dma_start(b1_sb[:], b1.ap())
        b2_sb = con.tile([1, D], F32)
        nc.sync.dma_start(b2_sb[:], b2.ap())
        ones_col = con.tile([128, 1], F32)
        nc.gpsimd.memset(ones_col[:], 1.0)

        txt0 = con.tile([128, D], F32)
        nc.sync.dma_start(txt0[:], text[0:128, :])
        txt1 = con.tile([128, D], F32)
        nc.sync.dma_start(txt1[0:C - 128, :], text[128:C, :])
        # textT[d, jd, c] = text[c, jd*128 + d]
        textT = con.tile([128, D // 128, C], F32)
        for jd in range(D // 128):
            pt = ps_wk.tile([128, 128], F32, tag="ps_wk")
            nc.tensor.transpose(pt[:, 0:128], txt0[:, jd * 128:(jd + 1) * 128],
                                identity[:])
            nc.vector.tensor_copy(textT[:, jd, 0:128], pt[:, 0:128])
            pt2 = ps_wk.tile([128, 128], F32, tag="ps_wk")
            nc.tensor.transpose(pt2[:, 0:C - 128],
                                txt1[0:C - 128, jd * 128:(jd + 1) * 128],
                                identity[0:C - 128, 0:C - 128])
            nc.vector.tensor_copy(textT[:, jd, 128:C], pt2[:, 0:C - 128])

        # ---- epilogue -------------------------------------------------------
        # vis = seg_sum / len
        vis = ep.tile([128, D], F32)
        nc.vector.tensor_scalar_mul(vis[:], psum_seg[:], recip_len[:])

        visT = ep.tile([128, D // 128, 128], F32)
        for jd in range(D // 128):
            pt = ps_wk.tile([128, 128], F32, tag="ps_wk")
            nc.tensor.transpose(pt[:], vis[:, jd * 128:(jd + 1) * 128], identity[:])
            nc.vector.tensor_copy(visT[:, jd, :], pt[:])

        h_ps = ps_mlp.tile([128, H], F32, tag="ps_mlp")
        for c in range(D // 128):
            nc.tensor.matmul(h_ps[:], visT[:, c, :], w1_sb[:, c, :],
                             start=(c == 0), stop=False)
        nc.tensor.matmul(h_ps[:], ones_row[:], b1_sb[:], start=False, stop=True)
        h_sb = ep.tile([128, H], F32)
        nc.vector.tensor_scalar_max(h_sb[:], h_ps[:], 0.0)

        hT = ep.tile([128, H // 128, 128], F32)
        for c in range(H // 128):
            pt = ps_wk.tile([128, 128], F32, tag="ps_wk")
            nc.tensor.transpose(pt[:], h_sb[:, c * 128:(c + 1) * 128], identity[:])
            nc.vector.tensor_copy(hT[:, c, :], pt[:])

        o_ps = ps_mlp.tile([128, D], F32, tag="ps_o")
        for c in range(H // 128):
            nc.tensor.matmul(o_ps[:], hT[:, c, :], w2_sb[:, c, :],
                             start=(c == 0), stop=False)
        nc.tensor.matmul(o_ps[:], ones_row[:], b2_sb[:], start=False, stop=True)

        ov = ep.tile([128, D], F32)
        nc.vector.tensor_tensor(ov[:], o_ps[:], vis[:], op=OP.add)

        # 1/||ov|| (the eps=1e-12 guard is vacuous at these magnitudes but free)
        sq = ep.tile([128, D], F32)
        ssq = ep.tile([128, 1], F32)
        nc.vector.scalar_tensor_tensor(sq[:], ov[:], 0.0, ov[:], op0=OP.add,
                                       op1=OP.mult, accum_out=ssq[:])
        nrm = ep.tile([128, 1], F32)
        nc.scalar.sqrt(nrm[:], ssq[:])
        nrm2 = ep.tile([128, 1], F32)
        nc.vector.tensor_scalar_max(nrm2[:], nrm[:], 1e-12)
        rnorm = ep.tile([128, 1], F32)
        nc.vector.reciprocal(rnorm[:], nrm2[:])

        ovT = ep.tile([128, D // 128, 128], F32)
        for jd in range(D // 128):
            pt = ps_wk.tile([128, 128], F32, tag="ps_wk")
            nc.tensor.transpose(pt[:], ov[:, jd * 128:(jd + 1) * 128], identity[:])
            nc.vector.tensor_copy(ovT[:, jd, :], pt[:])

        sim_ps = ps_mlp.tile([128, C], F32, tag="ps_sim")
        for c in range(D // 128):
            nc.tensor.matmul(sim_ps[:], ovT[:, c, :], textT[:, c, :],
                             start=(c == 0), stop=(c == D // 128 - 1))
        sim = ep.tile([128, C], F32)
        nc.vector.tensor_scalar_mul(sim[:], sim_ps[:], rnorm[:])

        # logits[p, k] = sim[p, idx_k[p]] via one-hot masked reduction:
        # junk = (iota_c == idx_k) * sim, logits_k = sum(junk) along free
        logits = ep.tile([128, 1 + NEG], F32)
        for k in range(1 + NEG):
            idx_ap = lab_col[:] if k == 0 else neg_col[:, k - 1:k]
            junk = ep.tile([128, C], F32, tag="junk")
            nc.vector.scalar_tensor_tensor(
                junk[:], iota_c_f[:], idx_ap, sim[:], op0=OP.is_equal,
                op1=OP.mult, accum_out=logits[:, k:k + 1])

        # loss terms: logsumexp(logits) - logits[:, 0]
        negmx = ep.tile([128, 1], F32)
        nc.vector.tensor_reduce(negmx[:], logits[:], axis=mybir.AxisListType.X,
                                op=OP.max, negate=True)
        exps = ep.tile([128, 1 + NEG], F32)
        sumexp = ep.tile([128, 1], F32)
        nc.scalar.activation(exps[:], logits[:], ACT.Exp, bias=negmx[:], scale=1.0,
                             accum_out=sumexp[:])
        lse = ep.tile([128, 1], F32)
        nc.scalar.activation(lse[:], sumexp[:], ACT.Ln)
        t1 = ep.tile([128, 1], F32)
        nc.vector.tensor_tensor(t1[:], lse[:], negmx[:], op=OP.subtract)
        term = ep.tile([128, 1], F32)
        nc.vector.tensor_tensor(term[:], t1[:], logits[:, 0:1], op=OP.subtract)

        loss_ps = ps_wk.tile([1, 1], F32, tag="ps_wk")
        nc.tensor.matmul(loss_ps[:], term[:], ones_col[:], start=True, stop=True)
        loss_sb = ep.tile([1, 1], F32)
        nc.vector.tensor_copy(loss_sb[:], loss_ps[:])
        nc.sync.dma_start(out[:], loss_sb[:])

    nc.compile()
    return nc


def prepare_shards(image_feat, seg_start, seg_len, t_len):
    """Per-core tile compaction. Returns (img_c list, shifts list, nt_pad)."""
    nb = image_feat.shape[0]
    tiles = [covered_tiles(seg_start[c], seg_len[c], t_len) for c in range(nb)]
    nt_pad = max(len(t) for t in tiles)
    nt_pad = ((nt_pad + GT - 1) // GT) * GT
    imgs, shs = [], []
    for c in range(nb):
        tc = tiles[c]
        img_c = np.zeros((nt_pad * TT, D), np.float32)
        sh = np.full((1, nt_pad), PAD_SHIFT, np.float32)
        src = np.asarray(image_feat[c])
        for i, o in enumerate(tc):
            r0, r1 = o * TT, min((o + 1) * TT, t_len)
            img_c[i * TT:i * TT + (r1 - r0)] = src[r0:r1]
            sh[0, i] = -float(TT * o)
        imgs.append(img_c)
        shs.append(sh)
    return imgs, shs, nt_pad


def make_in_maps(image_feat, text_feat, W1, b1, W2, b2, seg_start, seg_len,
                 labels, neg_idx, t_len=T):
    f32 = np.float32
    i32 = np.int32
    image_feat = np.asarray(image_feat)
    imgs, shs, nt_pad = prepare_shards(image_feat, np.asarray(seg_start),
                                       np.asarray(seg_len), t_len)
    nb = image_feat.shape[0]
    return [
        {
            "img": imgs[c],
            "shifts": shs[c],
            "text": np.ascontiguousarray(text_feat[c], dtype=f32),
            "w1": np.ascontiguousarray(W1, dtype=f32),
            "w2": np.ascontiguousarray(W2, dtype=f32),
            "b1": np.ascontiguousarray(b1, dtype=f32).reshape(1, H),
            "b2": np.ascontiguousarray(b2, dtype=f32).reshape(1, D),
            "seg_start": np.ascontiguousarray(seg_start[c], dtype=i32).reshape(1, P),
            "seg_len": np.ascontiguousarray(seg_len[c], dtype=i32).reshape(1, P),
            "labels": np.ascontiguousarray(labels[c], dtype=i32).reshape(1, P),
            "neg_idx": np.ascontiguousarray(neg_idx[c], dtype=i32).reshape(P, NEG),
        }
        for c in range(nb)
    ], nt_pad


_NC_CACHE = {}


def _get_nc(nt_pad):
    if nt_pad not in _NC_CACHE:
        _NC_CACHE[nt_pad] = build_kernel(nt_pad)
    return _NC_CACHE[nt_pad]


def kernel(image_feat, text_feat, W1, b1, W2, b2, seg_start, seg_len, labels,
           neg_idx, _trace=False):
    in_maps, nt_pad = make_in_maps(np.asarray(image_feat), np.asarray(text_feat),
                                   np.asarray(W1), np.asarray(b1), np.asarray(W2),
                                   np.asarray(b2), np.asarray(seg_start),
                                   np.asarray(seg_len), np.asarray(labels),
                                   np.asarray(neg_idx))
    nc = _get_nc(nt_pad)
    nb = np.asarray(image_feat).shape[0]
    res = run_bass_kernel_spmd(nc, in_maps, core_ids=list(range(nb)), trace=_trace)
    total = sum(float(res.results[c]["out"][0, 0]) for c in range(nb))
    loss = np.float32(total / (nb * P))
    if _trace:
        return loss, res
    return loss



# revision 3
# speedup vs baseline: 2.1629x; 2.1629x over previous
"""Trainium2 Bass kernel for nn_AnchorFreeHead (ragged segment mean-pool +
residual MLP + L2-normalize + contrastive CE loss).

Sharding: data-parallel over the batch (video) dim B=8 — one batch per
NeuronCore. FeatureProj weights and text_feat are replicated. Each core
computes the partial loss sum over its P=128 segments; the 8 partial sums are
averaged on the host (equivalent to the all-reduce of the scalar loss).

Algorithm (per core, batch b):
  Only rows that belong to at least one segment are shipped: the host gathers
  the sorted UNION of segment rows (~11.2k of 20000) into a dense, partition-
  major fp8(e4m3) buffer. Because the union is sorted and contains every
  segment row, each segment occupies a CONTIGUOUS RANK RANGE [rs_p, re_p) in
  it, so the 0/1 membership indicator for 128-row chunk k is
      ind_k[t, p] = (t - rs_p >= -128k) - (t - re_p >= -128k)
  built on-chip from two int16 compares + a subtract (grouped across chunks
  for DVE efficiency), entirely from the shipped [128,P] int16 rank offsets.
  seg_sum[p, d] then accumulates in PSUM via one indicator matmul per chunk
  (bf16 indicators x fp8 image rows). The epilogue (mean, residual MLP,
  L2-normalize, label/negative gather via one-hot reductions, logsumexp)
  runs mostly in bf16; fp8 image quantization costs ~3e-4 relative loss
  error (gate is 2e-2).
"""

import dataclasses
import numpy as np
from contextlib import ExitStack

import concourse.bass as bass
import concourse.tile as tile
from concourse import bacc, masks, mybir
from concourse.bass_utils import run_bass_kernel_spmd
import concourse.bass_utils as _bu

# NOTE: the baseline forced --enable-ldw-opt=true, but walrus rejects that
# optimization for bf16/fp8 LDWEIGHTS ("InstLdweights is not compatible with
# LDW optimization") — those dtypes take the FWL path instead, and the PE's
# 64-deep reorder window already pulls LDWEIGHTS ahead of in-flight MATMULs
# in silicon. So the default (ldw-opt off) is kept here.

F32 = mybir.dt.float32
BF16 = mybir.dt.bfloat16
FP8 = mybir.dt.float8e4
I16 = mybir.dt.int16
I32 = mybir.dt.int32
OP = mybir.AluOpType
ACT = mybir.ActivationFunctionType

B, T, D, P, C, NEG, H = 8, 20000, 512, 128, 200, 3, 256
TT = 128      # rows per chunk (matmul contraction)
GI = 16       # chunks per indicator vector-op group
GS = 8        # chunks per image DMA slice
PAD_NSH = -32000  # nsh for padding chunks: both compares true -> ind == 0

IMG_DT = FP8          # image payload dtype on device
IMG_NP = mybir.dt.np(FP8)
IND_DT = BF16         # indicator dtype (matmul lhsT)


def build_kernel(nt_pad):
    """Per-core Bass program over nt_pad compacted 128-row union chunks."""
    nc = bacc.Bacc("TRN2", target_bir_lowering=False, debug=False, num_devices=8)

    img = nc.dram_tensor("img", [TT, nt_pad * D], IMG_DT, kind="ExternalInput")
    cf1 = nc.dram_tensor("cf1", [TT, P], I16, kind="ExternalInput")
    cf2 = nc.dram_tensor("cf2", [TT, P], I16, kind="ExternalInput")
    nsh = nc.dram_tensor("nsh", [TT, nt_pad], I16, kind="ExternalInput")
    w1 = nc.dram_tensor("w1", [128, (D // 128) * H], BF16, kind="ExternalInput")
    w2 = nc.dram_tensor("w2", [128, (H // 128) * D], BF16, kind="ExternalInput")
    b1 = nc.dram_tensor("b1", [1, H], BF16, kind="ExternalInput")
    b2 = nc.dram_tensor("b2", [1, D], BF16, kind="ExternalInput")
    textT = nc.dram_tensor("textT", [128, (D // 128) * C], BF16, kind="ExternalInput")
    slen = nc.dram_tensor("slen", [P, 1], F32, kind="ExternalInput")
    lab = nc.dram_tensor("lab", [P, 1], F32, kind="ExternalInput")
    neg = nc.dram_tensor("neg_idx", [P, NEG], F32, kind="ExternalInput")
    out = nc.dram_tensor("out", [1, 1], F32, kind="ExternalOutput")

    assert nt_pad % GI == 0 and nt_pad % GS == 0
    n_gi, n_gs = nt_pad // GI, nt_pad // GS

    with tile.TileContext(nc) as tc, ExitStack() as ctx:
        con = ctx.enter_context(tc.tile_pool(name="con", bufs=1))
        ep = ctx.enter_context(tc.tile_pool(name="ep", bufs=1))
        gp = ctx.enter_context(tc.tile_pool(name="gp", bufs=2))
        ps_seg = ctx.enter_context(tc.tile_pool(name="ps_seg", bufs=1, space="PSUM"))
        ps_wk = ctx.enter_context(tc.tile_pool(name="ps_wk", bufs=2, space="PSUM"))
        ps_mlp = ctx.enter_context(tc.tile_pool(name="ps_mlp", bufs=1, space="PSUM"))

        # ---- scalar-engine act-table warmup (Exp/Ln/Sqrt), off critical path
        warm = con.tile([1, 1], F32)
        nc.gpsimd.memset(warm[:], 1.0)
        wo = con.tile([1, 1], F32)
        nc.scalar.activation(wo[:], warm[:], ACT.Exp)
        nc.scalar.activation(wo[:], warm[:], ACT.Ln)
        nc.scalar.activation(wo[:], warm[:], ACT.Sqrt)

        # ---- critical-path setup: rank-offset tensors ----------------------
        cf1_sb = con.tile([TT, P], I16)
        nc.sync.dma_start(cf1_sb[:], cf1.ap())
        cf2_sb = con.tile([TT, P], I16)
        nc.sync.dma_start(cf2_sb[:], cf2.ap())
        nsh_sb = con.tile([TT, nt_pad], I16)
        nc.sync.dma_start(nsh_sb[:], nsh.ap())

        def _bc_mid(ap, n):  # broadcast [128, X] -> [128, n, X]
            return dataclasses.replace(ap, ap=[ap.ap[0], [0, n], ap.ap[1]])

        def _bc_last(ap, n):  # broadcast [128, X] -> [128, X, n]
            return dataclasses.replace(ap, ap=[ap.ap[0], ap.ap[1], [0, n]])

        cf1_bc = _bc_mid(cf1_sb[:], GI)
        cf2_bc = _bc_mid(cf2_sb[:], GI)

        # ---- indicators: 3 grouped DVE passes per GI chunks ----------------
        ind_all = con.tile([TT, nt_pad, P], IND_DT)
        for g in range(n_gi):
            nsh_bc = _bc_last(nsh_sb[:, g * GI:(g + 1) * GI], P)
            g1 = gp.tile([TT, GI, P], I16, tag="g1")
            nc.vector.tensor_tensor(g1[:], cf1_bc, nsh_bc, op=OP.is_ge)
            g2 = gp.tile([TT, GI, P], I16, tag="g2")
            nc.vector.tensor_tensor(g2[:], cf2_bc, nsh_bc, op=OP.is_ge)
            nc.vector.tensor_tensor(ind_all[:, g * GI:(g + 1) * GI, :],
                                    g1[:], g2[:], op=OP.subtract)

        # ---- image stream + main indicator-matmul loop ---------------------
        img_sb = con.tile([TT, nt_pad, D], IMG_DT)
        psum_seg = ps_seg.tile([128, D], F32)
        for s in range(n_gs):
            nc.sync.dma_start(img_sb[:, s * GS:(s + 1) * GS, :],
                              img[:, s * GS * D:(s + 1) * GS * D])
            for j in range(GS):
                k = s * GS + j
                nc.tensor.matmul(psum_seg[:], ind_all[:, k, :], img_sb[:, k, :],
                                 start=(k == 0), stop=(k == nt_pad - 1))

        # ---- deferred setup (epilogue-only inputs) -------------------------
        identity = con.tile([128, 128], BF16)
        masks.make_identity(nc, identity[:])
        ones_row = con.tile([1, 128], BF16)
        nc.gpsimd.memset(ones_row[:], 1.0)
        ones_col = con.tile([128, 1], F32)
        nc.gpsimd.memset(ones_col[:], 1.0)

        sl_col = con.tile([P, 1], F32)
        nc.sync.dma_start(sl_col[:], slen.ap())
        recip_len = con.tile([P, 1], F32)
        nc.vector.reciprocal(recip_len[:], sl_col[:])
        lab_col = con.tile([P, 1], F32)
        nc.sync.dma_start(lab_col[:], lab.ap())
        neg_col = con.tile([P, NEG], F32)
        nc.sync.dma_start(neg_col[:], neg.ap())

        iota_c = con.tile([128, C], I32)
        nc.gpsimd.iota(iota_c[:], pattern=[[1, C]], base=0, channel_multiplier=0)
        iota_c_f = con.tile([128, C], F32)
        nc.vector.tensor_copy(iota_c_f[:], iota_c[:])

        w1_sb = con.tile([128, D // 128, H], BF16)
        nc.sync.dma_start(w1_sb[:], w1.ap())
        w2_sb = con.tile([128, H // 128, D], BF16)
        nc.sync.dma_start(w2_sb[:], w2.ap())
        b1_sb = con.tile([1, H], BF16)
        nc.sync.dma_start(b1_sb[:], b1.ap())
        b2_sb = con.tile([1, D], BF16)
        nc.sync.dma_start(b2_sb[:], b2.ap())
        textT_sb = con.tile([128, D // 128, C], BF16)
        nc.sync.dma_start(textT_sb[:], textT.ap())

        # ---- epilogue ------------------------------------------------------
        # vis = seg_sum / len  (bf16 copy; feeds both MLP input and residual)
        vis_b = ep.tile([128, D], BF16)
        nc.vector.tensor_scalar_mul(vis_b[:], psum_seg[:], recip_len[:])

        def transpose4(src, nblk, tag):
            dst = ep.tile([128, nblk, 128], BF16, tag=tag)
            for jd in range(nblk):
                pt = ps_wk.tile([128, 128], BF16, tag="ps_wk")
                nc.tensor.transpose(pt[:], src[:, jd * 128:(jd + 1) * 128],
                                    identity[:])
                nc.vector.tensor_copy(dst[:, jd, :], pt[:])
            return dst

        visT = transpose4(vis_b, D // 128, "visT")

        h_ps = ps_mlp.tile([128, H], F32, tag="ps_mlp")
        for c in range(D // 128):
            nc.tensor.matmul(h_ps[:], visT[:, c, :], w1_sb[:, c, :],
                             start=(c == 0), stop=False)
        nc.tensor.matmul(h_ps[:], ones_row[:], b1_sb[:], start=False, stop=True)
        h_sb = ep.tile([128, H], BF16)
        nc.vector.tensor_scalar_max(h_sb[:], h_ps[:], 0.0)

        hT = transpose4(h_sb, H // 128, "hT")

        o_ps = ps_mlp.tile([128, D], F32, tag="ps_o")
        for c in range(H // 128):
            nc.tensor.matmul(o_ps[:], hT[:, c, :], w2_sb[:, c, :],
                             start=(c == 0), stop=False)
        nc.tensor.matmul(o_ps[:], ones_row[:], b2_sb[:], start=False, stop=True)

        ov = ep.tile([128, D], BF16)
        nc.vector.tensor_tensor(ov[:], o_ps[:], vis_b[:], op=OP.add)

        # 1/||ov|| (the eps=1e-12 guard is vacuous at these magnitudes)
        sq = ep.tile([128, D], BF16)
        ssq = ep.tile([128, 1], F32)
        nc.vector.scalar_tensor_tensor(sq[:], ov[:], 0.0, ov[:], op0=OP.add,
                                       op1=OP.mult, accum_out=ssq[:])
        nrm = ep.tile([128, 1], F32)
        nc.scalar.sqrt(nrm[:], ssq[:])
        rnorm = ep.tile([128, 1], F32)
        nc.vector.reciprocal(rnorm[:], nrm[:])

        ovT = transpose4(ov, D // 128, "ovT")

        sim_ps = ps_mlp.tile([128, C], F32, tag="ps_sim")
        for c in range(D // 128):
            nc.tensor.matmul(sim_ps[:], ovT[:, c, :], textT_sb[:, c, :],
                             start=(c == 0), stop=(c == D // 128 - 1))

        # logits[p, k] = sim[p, idx_k[p]] via one-hot masked reduction
        logits = ep.tile([128, 1 + NEG], F32)
        for k in range(1 + NEG):
            idx_ap = lab_col[:] if k == 0 else neg_col[:, k - 1:k]
            junk = ep.tile([128, C], BF16, tag="junk")
            nc.vector.scalar_tensor_tensor(
                junk[:], iota_c_f[:], idx_ap, sim_ps[:], op0=OP.is_equal,
                op1=OP.mult, accum_out=logits[:, k:k + 1])
        logn = ep.tile([128, 1 + NEG], F32)
        nc.vector.tensor_scalar_mul(logn[:], logits[:], rnorm[:])

        # loss terms: logsumexp(logits) - logits[:, 0]
        negmx = ep.tile([128, 1], F32)
        nc.vector.tensor_reduce(negmx[:], logn[:], axis=mybir.AxisListType.X,
                                op=OP.max, negate=True)
        exps = ep.tile([128, 1 + NEG], F32)
        sumexp = ep.tile([128, 1], F32)
        nc.scalar.activation(exps[:], logn[:], ACT.Exp, bias=negmx[:], scale=1.0,
                             accum_out=sumexp[:])
        lse = ep.tile([128, 1], F32)
        nc.scalar.activation(lse[:], sumexp[:], ACT.Ln)
        t1 = ep.tile([128, 1], F32)
        nc.vector.tensor_tensor(t1[:], lse[:], negmx[:], op=OP.subtract)
        term = ep.tile([128, 1], F32)
        nc.vector.tensor_tensor(term[:], t1[:], logn[:, 0:1], op=OP.subtract)

        loss_ps = ps_wk.tile([1, 1], F32, tag="ps_loss")
        nc.tensor.matmul(loss_ps[:], term[:], ones_col[:], start=True, stop=True)
        loss_sb = ep.tile([1, 1], F32)
        nc.vector.tensor_copy(loss_sb[:], loss_ps[:])
        nc.sync.dma_start(out[:], loss_sb[:])

    nc.compile()
    return nc


def prepare_shards(image_feat, seg_start, seg_len):
    """Union-row compaction. Returns per-core (img, cf1, cf2, nsh) + nt_pad."""
    nb = image_feat.shape[0]
    rows_l, rs_l, re_l = [], [], []
    for b in range(nb):
        ss = seg_start[b].astype(np.int64)
        sl = seg_len[b].astype(np.int64)
        diff = np.zeros(T + 1, np.int32)
        np.add.at(diff, ss, 1)
        np.add.at(diff, ss + sl, -1)
        rows = np.flatnonzero(np.cumsum(diff[:-1]) > 0)
        rs = np.searchsorted(rows, ss)
        re_ = np.searchsorted(rows, ss + sl)
        assert (re_ - rs == sl).all()  # segment rows are contiguous ranks
        rows_l.append(rows)
        rs_l.append(rs)
        re_l.append(re_)
    nt = max((len(r) + TT - 1) // TT for r in rows_l)
    gl = np.lcm(GI, GS)
    nt_pad = ((nt + gl - 1) // gl) * gl

    shards = []
    t_iota = np.arange(TT, dtype=np.int64)
    for b in range(nb):
        rows = rows_l[b]
        gat = np.zeros((nt_pad * TT, D), IMG_NP)
        gat[:len(rows)] = image_feat[b][rows].astype(IMG_NP)
        img_pm = np.ascontiguousarray(
            gat.reshape(nt_pad, TT, D).transpose(1, 0, 2).reshape(TT, nt_pad * D))
        cf1 = (t_iota[:, None] - rs_l[b][None, :]).astype(np.int16)
        cf2 = (t_iota[:, None] - re_l[b][None, :]).astype(np.int16)
        nsh_row = np.full(nt_pad, PAD_NSH, np.int64)
        n_real = (len(rows) + TT - 1) // TT
        nsh_row[:n_real] = -TT * np.arange(n_real, dtype=np.int64)
        nsh = np.broadcast_to(nsh_row.astype(np.int16), (TT, nt_pad)).copy()
        shards.append((img_pm, cf1, cf2, nsh))
    return shards, nt_pad


def make_in_maps(image_feat, text_feat, W1, b1, W2, b2, seg_start, seg_len,
                 labels, neg_idx):
    bf = mybir.dt.np(BF16)
    shards, nt_pad = prepare_shards(image_feat, seg_start, seg_len)
    w1r = np.ascontiguousarray(
        W1.reshape(D // 128, 128, H).transpose(1, 0, 2).reshape(128, -1)).astype(bf)
    w2r = np.ascontiguousarray(
        W2.reshape(H // 128, 128, D).transpose(1, 0, 2).reshape(128, -1)).astype(bf)
    b1r = b1.reshape(1, H).astype(bf)
    b2r = b2.reshape(1, D).astype(bf)
    nb = image_feat.shape[0]
    maps = []
    for c in range(nb):
        img_pm, cf1, cf2, nsh = shards[c]
        ttr = np.ascontiguousarray(
            text_feat[c].T.reshape(D // 128, 128, C).transpose(1, 0, 2)
            .reshape(128, -1)).astype(bf)
        maps.append({
            "img": img_pm, "cf1": cf1, "cf2": cf2, "nsh": nsh,
            "w1": w1r, "w2": w2r, "b1": b1r, "b2": b2r, "textT": ttr,
            "slen": seg_len[c].astype(np.float32).reshape(P, 1),
            "lab": labels[c].astype(np.float32).reshape(P, 1),
            "neg_idx": neg_idx[c].astype(np.float32).reshape(P, NEG),
        })
    return maps, nt_pad


_NC_CACHE = {}


def _get_nc(nt_pad):
    if nt_pad not in _NC_CACHE:
        _NC_CACHE[nt_pad] = build_kernel(nt_pad)
    return _NC_CACHE[nt_pad]


def kernel(image_feat, text_feat, W1, b1, W2, b2, seg_start, seg_len, labels,
           neg_idx, _trace=False):
    in_maps, nt_pad = make_in_maps(
        np.asarray(image_feat), np.asarray(text_feat), np.asarray(W1),
        np.asarray(b1), np.asarray(W2), np.asarray(b2), np.asarray(seg_start),
        np.asarray(seg_len), np.asarray(labels), np.asarray(neg_idx))
    nc = _get_nc(nt_pad)
    nb = np.asarray(image_feat).shape[0]
    res = run_bass_kernel_spmd(nc, in_maps, core_ids=list(range(nb)), trace=_trace)
    total = sum(float(res.results[c]["out"][0, 0]) for c in range(nb))
    loss = np.float32(total / (nb * P))
    if _trace:
        return loss, res
    return loss


# revision 4
# speedup vs baseline: 2.5051x; 1.1582x over previous
"""Trainium2 Bass kernel for nn_AnchorFreeHead (ragged segment mean-pool +
residual MLP + L2-normalize + contrastive CE loss).

Sharding: data-parallel over the batch (video) dim B=8 — one batch per
NeuronCore. FeatureProj weights and text_feat are replicated. Each core
computes the partial loss sum over its P=128 segments; the 8 partial sums are
averaged on the host (equivalent to the all-reduce of the scalar loss).

Algorithm (per core, batch b):
  Only rows that belong to at least one segment are shipped: the host gathers
  the sorted UNION of segment rows (~11.2k of 20000) into a dense, partition-
  major fp8(e4m3) buffer. Because the union is sorted and contains every
  segment row, each segment occupies a CONTIGUOUS RANK RANGE [rs_p, re_p) in
  it, so the 0/1 membership indicator of row-rank r for segment p is
  (r >= rs_p) - (r >= re_p). With the shipped rank-offset tensor
  cf1M[t, k', p] = t + 128 k' - rs_p (k' = chunk-within-group), both compares
  for a whole group of GI chunks collapse to ONE tensor_scalar vs the
  immediate -(128*GI*g) — no stride-0 broadcast operand, so the DVE runs at
  its packed 16-bit rate. seg_sum[p, d] accumulates in PSUM via one
  indicator matmul per 128-row chunk (bf16 indicators x fp8 image rows).
  The epilogue (mean, residual MLP, L2-normalize, label/negative gather via
  one-hot reductions, logsumexp) runs mostly in bf16; fp8 image quantization
  costs ~3e-4 relative loss error (gate is 2e-2).
"""

import dataclasses
import numpy as np
from contextlib import ExitStack

import concourse.bass as bass
import concourse.tile as tile
from concourse import bacc, masks, mybir
from concourse.bass_utils import run_bass_kernel_spmd

F32 = mybir.dt.float32
BF16 = mybir.dt.bfloat16
FP8 = mybir.dt.float8e4
I16 = mybir.dt.int16
I32 = mybir.dt.int32
OP = mybir.AluOpType
ACT = mybir.ActivationFunctionType

B, T, D, P, C, NEG, H = 8, 20000, 512, 128, 200, 3, 256
TT = 128      # rows per chunk (matmul contraction)
GI = 16       # chunks per indicator vector-op group
GS = 8        # chunks per image DMA slice

IMG_DT = FP8
IMG_NP = mybir.dt.np(FP8)
BF16_NP = mybir.dt.np(BF16)
DOUBLE_ROW = False   # fp8 DoubleRow matmul pairs (requires IND_DT=FP8)
IND_DT = FP8 if DOUBLE_ROW else BF16


def build_kernel(nt_pad):
    """Per-core Bass program over nt_pad compacted 128-row union chunks."""
    nc = bacc.Bacc("TRN2", target_bir_lowering=False, debug=False, num_devices=8)

    img = nc.dram_tensor("img", [TT, nt_pad * D], IMG_DT, kind="ExternalInput")
    # cf1M [128, GI*P] | len_bc [128, P]
    meta16 = nc.dram_tensor("meta16", [TT, GI * P + P], I16, kind="ExternalInput")
    # w1 [128, 4*256] | w2 [128, 2*512] | textT [128, 4*200]
    NW1, NW2, NTX = (D // 128) * H, (H // 128) * D, (D // 128) * C
    wtx = nc.dram_tensor("wtx", [128, NW1 + NW2 + NTX], BF16, kind="ExternalInput")
    bias = nc.dram_tensor("bias", [1, H + D], BF16, kind="ExternalInput")
    # slen | lab | neg[3]
    meta32 = nc.dram_tensor("meta32", [P, 5], F32, kind="ExternalInput")
    out = nc.dram_tensor("out", [1, 1], F32, kind="ExternalOutput")

    assert nt_pad % GI == 0 and nt_pad % GS == 0
    n_gi, n_gs = nt_pad // GI, nt_pad // GS

    with tile.TileContext(nc) as tc, ExitStack() as ctx:
        con = ctx.enter_context(tc.tile_pool(name="con", bufs=1))
        ep = ctx.enter_context(tc.tile_pool(name="ep", bufs=1))
        gp = ctx.enter_context(tc.tile_pool(name="gp", bufs=2))
        ps_seg = ctx.enter_context(tc.tile_pool(name="ps_seg", bufs=1, space="PSUM"))
        ps_wk = ctx.enter_context(tc.tile_pool(name="ps_wk", bufs=2, space="PSUM"))
        ps_mlp = ctx.enter_context(tc.tile_pool(name="ps_mlp", bufs=1, space="PSUM"))

        # scalar-engine act-table warmup; tail order is Sqrt, Exp, Ln so end
        # the warmup on Sqrt (its reload is the only one this avoids).
        warm = con.tile([1, 1], F32)
        nc.gpsimd.memset(warm[:], 1.0)
        wo = con.tile([1, 1], F32)
        nc.scalar.activation(wo[:], warm[:], ACT.Exp)
        nc.scalar.activation(wo[:], warm[:], ACT.Ln)
        nc.scalar.activation(wo[:], warm[:], ACT.Sqrt)

        # ---- rank-offset tensors (one batched DMA) -------------------------
        m16 = con.tile([TT, GI * P + P], I16)
        nc.sync.dma_start(m16[:], meta16.ap())
        cf1M = m16[:, 0:GI * P].rearrange("t (k p) -> t k p", k=GI)
        len_ap = m16[:, GI * P:GI * P + P]
        len_bc = dataclasses.replace(
            len_ap, ap=[len_ap.ap[0], [0, GI], len_ap.ap[1]])
        cf2M = con.tile([TT, GI, P], I16)
        nc.vector.tensor_tensor(cf2M[:], cf1M, len_bc, op=OP.subtract)

        # ---- indicators: 2 grouped tensor_scalar + 1 subtract per GI chunks
        ind_all = con.tile([TT, nt_pad, P], IND_DT)
        for g in range(n_gi):
            sg = -(TT * GI) * g
            g1 = gp.tile([TT, GI, P], I16, tag="g1")
            nc.vector.tensor_scalar(g1[:], cf1M, sg, None, op0=OP.is_ge)
            g2 = gp.tile([TT, GI, P], I16, tag="g2")
            nc.vector.tensor_scalar(g2[:], cf2M[:], sg, None, op0=OP.is_ge)
            nc.vector.tensor_tensor(ind_all[:, g * GI:(g + 1) * GI, :],
                                    g1[:], g2[:], op=OP.subtract)

        # ---- image stream + main indicator-matmul loop ---------------------
        img_sb = con.tile([TT, nt_pad, D], IMG_DT)
        psum_seg = ps_seg.tile([128, D], F32)
        for s in range(n_gs):
            nc.sync.dma_start(img_sb[:, s * GS:(s + 1) * GS, :],
                              img[:, s * GS * D:(s + 1) * GS * D])
            if DOUBLE_ROW:
                for j in range(GS // 2):
                    k = s * GS + 2 * j
                    nc.tensor.matmul(
                        psum_seg[:], ind_all[:, k:k + 2, :], img_sb[:, k:k + 2, :],
                        start=(k == 0), stop=(k == nt_pad - 2),
                        perf_mode=mybir.MatmulPerfMode.DoubleRow)
            else:
                for j in range(GS):
                    k = s * GS + j
                    nc.tensor.matmul(psum_seg[:], ind_all[:, k, :],
                                     img_sb[:, k, :],
                                     start=(k == 0), stop=(k == nt_pad - 1))

        # ---- deferred setup (epilogue-only inputs) -------------------------
        identity = con.tile([128, 128], BF16)
        masks.make_identity(nc, identity[:])
        ones_row = con.tile([1, 128], BF16)
        nc.gpsimd.memset(ones_row[:], 1.0)
        ones_col = con.tile([128, 1], F32)
        nc.gpsimd.memset(ones_col[:], 1.0)

        wtx_sb = con.tile([128, NW1 + NW2 + NTX], BF16)
        nc.sync.dma_start(wtx_sb[:], wtx.ap())
        w1_sb = wtx_sb[:, 0:NW1].rearrange("t (c h) -> t c h", c=D // 128)
        w2_sb = wtx_sb[:, NW1:NW1 + NW2].rearrange("t (c h) -> t c h", c=H // 128)
        textT_sb = wtx_sb[:, NW1 + NW2:].rearrange("t (c h) -> t c h", c=D // 128)
        bias_sb = con.tile([1, H + D], BF16)
        nc.sync.dma_start(bias_sb[:], bias.ap())
        b1_sb, b2_sb = bias_sb[:, 0:H], bias_sb[:, H:]

        m32 = con.tile([P, 5], F32)
        nc.sync.dma_start(m32[:], meta32.ap())
        recip_len = con.tile([P, 1], F32)
        nc.vector.reciprocal(recip_len[:], m32[:, 0:1])
        idx_bf = con.tile([P, 1 + NEG], BF16)
        nc.vector.tensor_copy(idx_bf[:], m32[:, 1:5])

        iota_c = con.tile([128, C], I32)
        nc.gpsimd.iota(iota_c[:], pattern=[[1, C]], base=0, channel_multiplier=0)
        iota_bf = con.tile([128, C], BF16)
        nc.vector.tensor_copy(iota_bf[:], iota_c[:])

        # ---- epilogue ------------------------------------------------------
        vis_b = ep.tile([128, D], BF16)
        nc.vector.tensor_scalar_mul(vis_b[:], psum_seg[:], recip_len[:])

        def transposeN(src, nblk, tag):
            dst = ep.tile([128, nblk, 128], BF16, tag=tag)
            for jd in range(nblk):
                pt = ps_wk.tile([128, 128], BF16, tag="ps_wk")
                nc.tensor.transpose(pt[:], src[:, jd * 128:(jd + 1) * 128],
                                    identity[:])
                nc.vector.tensor_copy(dst[:, jd, :], pt[:])
            return dst

        visT = transposeN(vis_b, D // 128, "visT")

        h_ps = ps_mlp.tile([128, H], F32, tag="ps_mlp")
        for c in range(D // 128):
            nc.tensor.matmul(h_ps[:], visT[:, c, :], w1_sb[:, c, :],
                             start=(c == 0), stop=False)
        nc.tensor.matmul(h_ps[:], ones_row[:], b1_sb, start=False, stop=True)
        h_sb = ep.tile([128, H], BF16)
        nc.vector.tensor_scalar_max(h_sb[:], h_ps[:], 0.0)

        hT = transposeN(h_sb, H // 128, "hT")

        o_ps = ps_mlp.tile([128, D], F32, tag="ps_o")
        for c in range(H // 128):
            nc.tensor.matmul(o_ps[:], hT[:, c, :], w2_sb[:, c, :],
                             start=(c == 0), stop=False)
        nc.tensor.matmul(o_ps[:], ones_row[:], b2_sb, start=False, stop=True)

        ov = ep.tile([128, D], BF16)
        nc.vector.tensor_tensor(ov[:], o_ps[:], vis_b[:], op=OP.add)

        # 1/||ov|| (the eps=1e-12 guard is vacuous at these magnitudes)
        sq = ep.tile([128, D], BF16)
        ssq = ep.tile([128, 1], F32)
        nc.vector.scalar_tensor_tensor(sq[:], ov[:], 0.0, ov[:], op0=OP.add,
                                       op1=OP.mult, accum_out=ssq[:])
        nrm = ep.tile([128, 1], F32)
        nc.scalar.sqrt(nrm[:], ssq[:])
        rnorm = ep.tile([128, 1], F32)
        nc.vector.reciprocal(rnorm[:], nrm[:])

        ovT = transposeN(ov, D // 128, "ovT")

        sim_ps = ps_mlp.tile([128, C], F32, tag="ps_sim")
        for c in range(D // 128):
            nc.tensor.matmul(sim_ps[:], ovT[:, c, :], textT_sb[:, c, :],
                             start=(c == 0), stop=(c == D // 128 - 1))
        sim_sb = ep.tile([128, C], BF16)
        nc.vector.tensor_copy(sim_sb[:], sim_ps[:])

        # logits[p, k] = sim[p, idx_k[p]] via one-hot masked reduction
        logits = ep.tile([128, 1 + NEG], F32)
        for k in range(1 + NEG):
            junk = ep.tile([128, C], BF16, tag="junk")
            nc.vector.scalar_tensor_tensor(
                junk[:], iota_bf[:], idx_bf[:, k:k + 1], sim_sb[:],
                op0=OP.is_equal, op1=OP.mult, accum_out=logits[:, k:k + 1])
        logn = ep.tile([128, 1 + NEG], F32)
        nc.vector.tensor_scalar_mul(logn[:], logits[:], rnorm[:])

        # loss terms: logsumexp(logits) - logits[:, 0]; |logits| <= ~6 so the
        # max-shift of the reference logsumexp is skipped (exp stays finite).
        exps = ep.tile([128, 1 + NEG], F32)
        sumexp = ep.tile([128, 1], F32)
        nc.scalar.activation(exps[:], logn[:], ACT.Exp, accum_out=sumexp[:])
        lse = ep.tile([128, 1], F32)
        nc.scalar.activation(lse[:], sumexp[:], ACT.Ln)
        term = ep.tile([128, 1], F32)
        nc.vector.tensor_tensor(term[:], lse[:], logn[:, 0:1], op=OP.subtract)

        loss_ps = ps_wk.tile([1, 1], F32, tag="ps_loss")
        nc.tensor.matmul(loss_ps[:], term[:], ones_col[:], start=True, stop=True)
        loss_sb = ep.tile([1, 1], F32)
        nc.vector.tensor_copy(loss_sb[:], loss_ps[:])
        nc.sync.dma_start(out[:], loss_sb[:])

    nc.compile()
    return nc


def prepare_shards(image_feat, seg_start, seg_len):
    """Union-row compaction. Returns per-core (img, meta16) + nt_pad."""
    nb = image_feat.shape[0]
    rows_l, rs_l, re_l = [], [], []
    for b in range(nb):
        ss = seg_start[b].astype(np.int64)
        sl = seg_len[b].astype(np.int64)
        diff = np.zeros(T + 1, np.int32)
        np.add.at(diff, ss, 1)
        np.add.at(diff, ss + sl, -1)
        rows = np.flatnonzero(np.cumsum(diff[:-1]) > 0)
        rs = np.searchsorted(rows, ss)
        re_ = np.searchsorted(rows, ss + sl)
        assert (re_ - rs == sl).all()  # segment rows are contiguous ranks
        rows_l.append(rows)
        rs_l.append(rs)
        re_l.append(re_)
    nt = max((len(r) + TT - 1) // TT for r in rows_l)
    gl = np.lcm(GI, GS)
    nt_pad = ((nt + gl - 1) // gl) * gl

    shards = []
    # cf1M[t, k', p] = t + 128*k' - rs_p ; len_bc[t, p] = seg_len_p
    tk = (np.arange(TT)[:, None] + TT * np.arange(GI)[None, :]).reshape(
        TT, GI, 1).astype(np.int64)
    for b in range(nb):
        rows = rows_l[b]
        gat = np.zeros((nt_pad * TT, D), IMG_NP)
        gat[:len(rows)] = image_feat[b][rows].astype(IMG_NP)
        img_pm = np.ascontiguousarray(
            gat.reshape(nt_pad, TT, D).transpose(1, 0, 2).reshape(TT, nt_pad * D))
        cf1M = (tk - rs_l[b].reshape(1, 1, P)).reshape(TT, GI * P)
        lenb = np.broadcast_to((re_l[b] - rs_l[b]).reshape(1, P), (TT, P))
        m16 = np.concatenate([cf1M, lenb], axis=1).astype(np.int16)
        shards.append((img_pm, np.ascontiguousarray(m16)))
    return shards, nt_pad


def make_in_maps(image_feat, text_feat, W1, b1, W2, b2, seg_start, seg_len,
                 labels, neg_idx):
    shards, nt_pad = prepare_shards(image_feat, seg_start, seg_len)
    w1r = W1.reshape(D // 128, 128, H).transpose(1, 0, 2).reshape(128, -1)
    w2r = W2.reshape(H // 128, 128, D).transpose(1, 0, 2).reshape(128, -1)
    biasr = np.concatenate([b1, b2]).reshape(1, H + D).astype(BF16_NP)
    nb = image_feat.shape[0]
    maps = []
    for c in range(nb):
        img_pm, m16 = shards[c]
        ttr = text_feat[c].T.reshape(D // 128, 128, C).transpose(1, 0, 2).reshape(
            128, -1)
        wtx = np.ascontiguousarray(
            np.concatenate([w1r, w2r, ttr], axis=1)).astype(BF16_NP)
        m32 = np.concatenate([
            seg_len[c].reshape(P, 1), labels[c].reshape(P, 1),
            neg_idx[c].reshape(P, NEG)], axis=1).astype(np.float32)
        maps.append({"img": img_pm, "meta16": m16, "wtx": wtx, "bias": biasr,
                     "meta32": m32})
    return maps, nt_pad


_NC_CACHE = {}


def _get_nc(nt_pad):
    if nt_pad not in _NC_CACHE:
        _NC_CACHE[nt_pad] = build_kernel(nt_pad)
    return _NC_CACHE[nt_pad]


def kernel(image_feat, text_feat, W1, b1, W2, b2, seg_start, seg_len, labels,
           neg_idx, _trace=False):
    in_maps, nt_pad = make_in_maps(
        np.asarray(image_feat), np.asarray(text_feat), np.asarray(W1),
        np.asarray(b1), np.asarray(W2), np.asarray(b2), np.asarray(seg_start),
        np.asarray(seg_len), np.asarray(labels), np.asarray(neg_idx))
    nc = _get_nc(nt_pad)
    nb = np.asarray(image_feat).shape[0]
    res = run_bass_kernel_spmd(nc, in_maps, core_ids=list(range(nb)), trace=_trace)
    total = sum(float(res.results[c]["out"][0, 0]) for c in range(nb))
    loss = np.float32(total / (nb * P))
    if _trace:
        return loss, res
    return loss


# revision 6
# speedup vs baseline: 2.7027x; 1.0788x over previous
"""Trainium2 Bass kernel for nn_AnchorFreeHead (ragged segment mean-pool +
residual MLP + L2-normalize + contrastive CE loss).

Sharding: data-parallel over the batch (video) dim B=8 — one batch per
NeuronCore. FeatureProj weights and text_feat are replicated. Each core
computes the partial loss sum over its P=128 segments; the 8 partial sums are
averaged on the host (equivalent to the all-reduce of the scalar loss).

Algorithm (per core, batch b):
  Only rows that belong to at least one segment are shipped: the host gathers
  the sorted UNION of segment rows (~11.2k of 20000) into a dense, partition-
  major fp8(e4m3) buffer. Because the union is sorted and contains every
  segment row, each segment occupies a CONTIGUOUS RANK RANGE [rs_p, re_p) in
  it, so the 0/1 membership indicator of row-rank r for segment p is
  (r >= rs_p) - (r >= re_p). With the shipped rank-offset tensor
  cf1M[t, k', p] = t + 128 k' - rs_p (k' = chunk-within-group), both compares
  for a whole group of GI chunks collapse to ONE tensor_scalar vs the
  immediate -(128*GI*g) — no stride-0 broadcast operand, so the DVE runs at
  its packed 16-bit rate. seg_sum[p, d] accumulates in PSUM via one
  indicator matmul per 128-row chunk (bf16 indicators x fp8 image rows).
  The epilogue (mean, residual MLP, L2-normalize, label/negative gather via
  one-hot reductions, logsumexp) runs mostly in bf16; fp8 image quantization
  costs ~3e-4 relative loss error (gate is 2e-2).
"""

import dataclasses
import numpy as np
from contextlib import ExitStack

import concourse.bass as bass
import concourse.tile as tile
from concourse import bacc, masks, mybir
from concourse.bass_utils import run_bass_kernel_spmd

F32 = mybir.dt.float32
BF16 = mybir.dt.bfloat16
FP8 = mybir.dt.float8e4
I16 = mybir.dt.int16
I32 = mybir.dt.int32
OP = mybir.AluOpType
ACT = mybir.ActivationFunctionType

B, T, D, P, C, NEG, H = 8, 20000, 512, 128, 200, 3, 256
TT = 128      # rows per chunk (matmul contraction)
GI = 16       # chunks per indicator vector-op group
GS = 8        # chunks per image DMA slice

IMG_DT = FP8
IMG_NP = mybir.dt.np(FP8)
BF16_NP = mybir.dt.np(BF16)
DOUBLE_ROW = True    # fp8 DoubleRow matmul pairs (requires IND_DT=FP8)
IND_DT = FP8 if DOUBLE_ROW else BF16


def build_kernel(nt_pad):
    """Per-core Bass program over nt_pad compacted 128-row union chunks."""
    nc = bacc.Bacc("TRN2", target_bir_lowering=False, debug=False, num_devices=8)

    img = nc.dram_tensor("img", [TT, nt_pad * D], IMG_DT, kind="ExternalInput")
    # cf1M [128, GI*P] | len_bc [128, P]
    meta16 = nc.dram_tensor("meta16", [TT, GI * P + P], I16, kind="ExternalInput")
    # w1 [128, 4*256] | w2 [128, 2*512] | textT [128, 4*200]
    NW1, NW2, NTX = (D // 128) * H, (H // 128) * D, (D // 128) * C
    wtx = nc.dram_tensor("wtx", [128, NW1 + NW2 + NTX], BF16, kind="ExternalInput")
    bias = nc.dram_tensor("bias", [1, H + D], BF16, kind="ExternalInput")
    # slen | lab | neg[3]
    meta32 = nc.dram_tensor("meta32", [P, 5], F32, kind="ExternalInput")
    out = nc.dram_tensor("out", [1, 1], F32, kind="ExternalOutput")

    assert nt_pad % GI == 0 and nt_pad % GS == 0
    n_gi, n_gs = nt_pad // GI, nt_pad // GS

    with tile.TileContext(nc) as tc, ExitStack() as ctx:
        con = ctx.enter_context(tc.tile_pool(name="con", bufs=1))
        ep = ctx.enter_context(tc.tile_pool(name="ep", bufs=1))
        gp = ctx.enter_context(tc.tile_pool(name="gp", bufs=2))
        ps_seg = ctx.enter_context(tc.tile_pool(name="ps_seg", bufs=1, space="PSUM"))
        ps_wk = ctx.enter_context(tc.tile_pool(name="ps_wk", bufs=2, space="PSUM"))
        ps_mlp = ctx.enter_context(tc.tile_pool(name="ps_mlp", bufs=1, space="PSUM"))

        # scalar-engine act-table warmup; tail order is Sqrt, Exp, Ln so end
        # the warmup on Sqrt (its reload is the only one this avoids).
        warm = con.tile([1, 1], F32)
        nc.gpsimd.memset(warm[:], 1.0)
        wo = con.tile([1, 1], F32)
        nc.scalar.activation(wo[:], warm[:], ACT.Exp)
        nc.scalar.activation(wo[:], warm[:], ACT.Ln)
        nc.scalar.activation(wo[:], warm[:], ACT.Sqrt)

        # ---- rank-offset tensors (one batched DMA) -------------------------
        m16 = con.tile([TT, GI * P + P], I16)
        nc.sync.dma_start(m16[:], meta16.ap())
        cf1M = m16[:, 0:GI * P].rearrange("t (k p) -> t k p", k=GI)
        len_ap = m16[:, GI * P:GI * P + P]
        len_bc = dataclasses.replace(
            len_ap, ap=[len_ap.ap[0], [0, GI], len_ap.ap[1]])
        cf2M = con.tile([TT, GI, P], I16)
        nc.vector.tensor_tensor(cf2M[:], cf1M, len_bc, op=OP.subtract)

        # ---- indicators: 2 grouped tensor_scalar + 1 subtract per GI chunks
        ind_all = con.tile([TT, nt_pad, P], IND_DT)
        for g in range(n_gi):
            sg = -(TT * GI) * g
            g1 = gp.tile([TT, GI, P], I16, tag="g1")
            nc.vector.tensor_scalar(g1[:], cf1M, sg, None, op0=OP.is_ge)
            g2 = gp.tile([TT, GI, P], I16, tag="g2")
            nc.vector.tensor_scalar(g2[:], cf2M[:], sg, None, op0=OP.is_ge)
            nc.vector.tensor_tensor(ind_all[:, g * GI:(g + 1) * GI, :],
                                    g1[:], g2[:], op=OP.subtract)

        # ---- image stream + main indicator-matmul loop ---------------------
        img_sb = con.tile([TT, nt_pad, D], IMG_DT)
        psum_seg = ps_seg.tile([128, D], F32)
        for s in range(n_gs):
            nc.sync.dma_start(img_sb[:, s * GS:(s + 1) * GS, :],
                              img[:, s * GS * D:(s + 1) * GS * D])
            if DOUBLE_ROW:
                for j in range(GS // 2):
                    k = s * GS + 2 * j
                    nc.tensor.matmul(
                        psum_seg[:], ind_all[:, k:k + 2, :], img_sb[:, k:k + 2, :],
                        start=(k == 0), stop=(k == nt_pad - 2),
                        perf_mode=mybir.MatmulPerfMode.DoubleRow)
            else:
                for j in range(GS):
                    k = s * GS + j
                    nc.tensor.matmul(psum_seg[:], ind_all[:, k, :],
                                     img_sb[:, k, :],
                                     start=(k == 0), stop=(k == nt_pad - 1))

        # ---- deferred setup (epilogue-only inputs) -------------------------
        identity = con.tile([128, 128], BF16)
        masks.make_identity(nc, identity[:])
        ones_row = con.tile([1, 128], BF16)
        nc.gpsimd.memset(ones_row[:], 1.0)
        ones_col = con.tile([128, 1], F32)
        nc.gpsimd.memset(ones_col[:], 1.0)

        wtx_sb = con.tile([128, NW1 + NW2 + NTX], BF16)
        nc.sync.dma_start(wtx_sb[:], wtx.ap())
        w1_sb = wtx_sb[:, 0:NW1].rearrange("t (c h) -> t c h", c=D // 128)
        w2_sb = wtx_sb[:, NW1:NW1 + NW2].rearrange("t (c h) -> t c h", c=H // 128)
        textT_sb = wtx_sb[:, NW1 + NW2:].rearrange("t (c h) -> t c h", c=D // 128)
        bias_sb = con.tile([1, H + D], BF16)
        nc.sync.dma_start(bias_sb[:], bias.ap())
        b1_sb, b2_sb = bias_sb[:, 0:H], bias_sb[:, H:]

        m32 = con.tile([P, 5], F32)
        nc.sync.dma_start(m32[:], meta32.ap())
        recip_len = con.tile([P, 1], F32)
        nc.vector.reciprocal(recip_len[:], m32[:, 0:1])
        idx_bf = con.tile([P, 1 + NEG], BF16)
        nc.vector.tensor_copy(idx_bf[:], m32[:, 1:5])

        iota_c = con.tile([128, C], I32)
        nc.gpsimd.iota(iota_c[:], pattern=[[1, C]], base=0, channel_multiplier=0)
        iota_bf = con.tile([128, C], BF16)
        nc.vector.tensor_copy(iota_bf[:], iota_c[:])

        # ---- epilogue ------------------------------------------------------
        vis_b = ep.tile([128, D], BF16)
        nc.vector.tensor_scalar_mul(vis_b[:], psum_seg[:], recip_len[:])

        def transposeN(src, nblk, tag):
            dst = ep.tile([128, nblk, 128], BF16, tag=tag)
            for jd in range(nblk):
                pt = ps_wk.tile([128, 128], BF16, tag="ps_wk")
                nc.tensor.transpose(pt[:], src[:, jd * 128:(jd + 1) * 128],
                                    identity[:])
                nc.vector.tensor_copy(dst[:, jd, :], pt[:])
            return dst

        visT = transposeN(vis_b, D // 128, "visT")

        h_ps = ps_mlp.tile([128, H], F32, tag="ps_mlp")
        for c in range(D // 128):
            nc.tensor.matmul(h_ps[:], visT[:, c, :], w1_sb[:, c, :],
                             start=(c == 0), stop=False)
        nc.tensor.matmul(h_ps[:], ones_row[:], b1_sb, start=False, stop=True)
        h_sb = ep.tile([128, H], BF16)
        nc.vector.tensor_scalar_max(h_sb[:], h_ps[:], 0.0)

        hT = transposeN(h_sb, H // 128, "hT")

        o_ps = ps_mlp.tile([128, D], F32, tag="ps_o")
        for c in range(H // 128):
            nc.tensor.matmul(o_ps[:], hT[:, c, :], w2_sb[:, c, :],
                             start=(c == 0), stop=False)
        nc.tensor.matmul(o_ps[:], ones_row[:], b2_sb, start=False, stop=True)

        ov = ep.tile([128, D], BF16)
        nc.vector.tensor_tensor(ov[:], o_ps[:], vis_b[:], op=OP.add)

        # 1/||ov|| (the eps=1e-12 guard is vacuous at these magnitudes)
        sq = ep.tile([128, D], BF16)
        ssq = ep.tile([128, 1], F32)
        nc.vector.scalar_tensor_tensor(sq[:], ov[:], 0.0, ov[:], op0=OP.add,
                                       op1=OP.mult, accum_out=ssq[:])
        nrm = ep.tile([128, 1], F32)
        nc.scalar.sqrt(nrm[:], ssq[:])
        rnorm = ep.tile([128, 1], F32)
        nc.vector.reciprocal(rnorm[:], nrm[:])

        ovT = transposeN(ov, D // 128, "ovT")

        sim_ps = ps_mlp.tile([128, C], F32, tag="ps_sim")
        for c in range(D // 128):
            nc.tensor.matmul(sim_ps[:], ovT[:, c, :], textT_sb[:, c, :],
                             start=(c == 0), stop=(c == D // 128 - 1))
        # fold the 1/||ov|| normalize into the PSUM->SBUF copy of sim
        sim_sb = ep.tile([128, C], BF16)
        nc.vector.tensor_scalar_mul(sim_sb[:], sim_ps[:], rnorm[:])

        # logits[p, k] = sim[p, idx_k[p]] via one-hot masked reduction
        logits = ep.tile([128, 1 + NEG], F32)
        for k in range(1 + NEG):
            junk = ep.tile([128, C], BF16, tag="junk")
            nc.vector.scalar_tensor_tensor(
                junk[:], iota_bf[:], idx_bf[:, k:k + 1], sim_sb[:],
                op0=OP.is_equal, op1=OP.mult, accum_out=logits[:, k:k + 1])

        # loss terms: logsumexp(logits) - logits[:, 0]; |logits| <= ~6 so the
        # max-shift of the reference logsumexp is skipped (exp stays finite).
        exps = ep.tile([128, 1 + NEG], F32)
        sumexp = ep.tile([128, 1], F32)
        nc.scalar.activation(exps[:], logits[:], ACT.Exp, accum_out=sumexp[:])
        lse = ep.tile([128, 1], F32)
        nc.scalar.activation(lse[:], sumexp[:], ACT.Ln)
        term = ep.tile([128, 1], F32)
        nc.vector.tensor_tensor(term[:], lse[:], logits[:, 0:1], op=OP.subtract)

        loss_ps = ps_wk.tile([1, 1], F32, tag="ps_loss")
        nc.tensor.matmul(loss_ps[:], term[:], ones_col[:], start=True, stop=True)
        loss_sb = ep.tile([1, 1], F32)
        nc.vector.tensor_copy(loss_sb[:], loss_ps[:])
        nc.sync.dma_start(out[:], loss_sb[:])

    nc.compile()
    return nc


def prepare_shards(image_feat, seg_start, seg_len):
    """Union-row compaction. Returns per-core (img, meta16) + nt_pad."""
    nb = image_feat.shape[0]
    rows_l, rs_l, re_l = [], [], []
    for b in range(nb):
        ss = seg_start[b].astype(np.int64)
        sl = seg_len[b].astype(np.int64)
        diff = np.zeros(T + 1, np.int32)
        np.add.at(diff, ss, 1)
        np.add.at(diff, ss + sl, -1)
        rows = np.flatnonzero(np.cumsum(diff[:-1]) > 0)
        rs = np.searchsorted(rows, ss)
        re_ = np.searchsorted(rows, ss + sl)
        assert (re_ - rs == sl).all()  # segment rows are contiguous ranks
        rows_l.append(rows)
        rs_l.append(rs)
        re_l.append(re_)
    nt = max((len(r) + TT - 1) // TT for r in rows_l)
    gl = np.lcm(GI, GS)
    nt_pad = ((nt + gl - 1) // gl) * gl

    shards = []
    # cf1M[t, k', p] = t + 128*k' - rs_p ; len_bc[t, p] = seg_len_p
    tk = (np.arange(TT)[:, None] + TT * np.arange(GI)[None, :]).reshape(
        TT, GI, 1).astype(np.int64)
    for b in range(nb):
        rows = rows_l[b]
        gat = np.zeros((nt_pad * TT, D), IMG_NP)
        gat[:len(rows)] = image_feat[b][rows].astype(IMG_NP)
        img_pm = np.ascontiguousarray(
            gat.reshape(nt_pad, TT, D).transpose(1, 0, 2).reshape(TT, nt_pad * D))
        cf1M = (tk - rs_l[b].reshape(1, 1, P)).reshape(TT, GI * P)
        lenb = np.broadcast_to((re_l[b] - rs_l[b]).reshape(1, P), (TT, P))
        m16 = np.concatenate([cf1M, lenb], axis=1).astype(np.int16)
        shards.append((img_pm, np.ascontiguousarray(m16)))
    return shards, nt_pad


def make_in_maps(image_feat, text_feat, W1, b1, W2, b2, seg_start, seg_len,
                 labels, neg_idx):
    shards, nt_pad = prepare_shards(image_feat, seg_start, seg_len)
    w1r = W1.reshape(D // 128, 128, H).transpose(1, 0, 2).reshape(128, -1)
    w2r = W2.reshape(H // 128, 128, D).transpose(1, 0, 2).reshape(128, -1)
    biasr = np.concatenate([b1, b2]).reshape(1, H + D).astype(BF16_NP)
    nb = image_feat.shape[0]
    maps = []
    for c in range(nb):
        img_pm, m16 = shards[c]
        ttr = text_feat[c].T.reshape(D // 128, 128, C).transpose(1, 0, 2).reshape(
            128, -1)
        wtx = np.ascontiguousarray(
            np.concatenate([w1r, w2r, ttr], axis=1)).astype(BF16_NP)
        m32 = np.concatenate([
            seg_len[c].reshape(P, 1), labels[c].reshape(P, 1),
            neg_idx[c].reshape(P, NEG)], axis=1).astype(np.float32)
        maps.append({"img": img_pm, "meta16": m16, "wtx": wtx, "bias": biasr,
                     "meta32": m32})
    return maps, nt_pad


_NC_CACHE = {}


def _get_nc(nt_pad):
    if nt_pad not in _NC_CACHE:
        _NC_CACHE[nt_pad] = build_kernel(nt_pad)
    return _NC_CACHE[nt_pad]


def kernel(image_feat, text_feat, W1, b1, W2, b2, seg_start, seg_len, labels,
           neg_idx, _trace=False):
    in_maps, nt_pad = make_in_maps(
        np.asarray(image_feat), np.asarray(text_feat), np.asarray(W1),
        np.asarray(b1), np.asarray(W2), np.asarray(b2), np.asarray(seg_start),
        np.asarray(seg_len), np.asarray(labels), np.asarray(neg_idx))
    nc = _get_nc(nt_pad)
    nb = np.asarray(image_feat).shape[0]
    res = run_bass_kernel_spmd(nc, in_maps, core_ids=list(range(nb)), trace=_trace)
    total = sum(float(res.results[c]["out"][0, 0]) for c in range(nb))
    loss = np.float32(total / (nb * P))
    if _trace:
        return loss, res
    return loss


# revision 17
# speedup vs baseline: 2.8487x; 1.0540x over previous
"""Trainium2 Bass kernel for nn_AnchorFreeHead (ragged segment mean-pool +
residual MLP + L2-normalize + contrastive CE loss).

Sharding: data-parallel over the batch (video) dim B=8 — one batch per
NeuronCore. FeatureProj weights and text_feat are replicated. Each core
computes the partial loss sum over its P=128 segments; the 8 partial sums are
averaged on the host (equivalent to the all-reduce of the scalar loss).

Algorithm (per core, batch b):
  Only rows that belong to at least one segment are shipped: the host gathers
  the sorted UNION of segment rows (~11.2k of 20000) into a dense, partition-
  major fp8(e4m3) buffer. Because the union is sorted and contains every
  segment row, each segment occupies a CONTIGUOUS RANK RANGE [rs_p, re_p) in
  it, so the 0/1 membership indicator of row-rank r for segment p is
  (r >= rs_p) - (r >= re_p). With the shipped rank-offset tensor
  cf1M[t, k', p] = t + 128 k' - rs_p (int16, k' = chunk-within-group) and
  cf2M = cf1M - len, the indicators for a whole group of GI chunks are TWO
  fused DVE ops against the immediate sg = -128*GI*g:
      a = (cf2M >= sg);  ind = (cf1M >= sg) - a
  — every tensor operand has unit innermost stride, so the DVE runs at its
  packed 16-bit rate. seg_sum[p, d] accumulates in PSUM via one fp8
  DoubleRow matmul per pair of 128-row chunks.
  The epilogue (mean, residual MLP, L2-normalize, label/negative gather via
  one-hot reductions, logsumexp) runs mostly in bf16; fp8 image quantization
  costs ~3e-4 relative loss error (gate is 2e-2).
"""

import dataclasses
import numpy as np
from contextlib import ExitStack

import concourse.bass as bass
import concourse.tile as tile
from concourse import bacc, masks, mybir
from concourse.bass_utils import run_bass_kernel_spmd

F32 = mybir.dt.float32
BF16 = mybir.dt.bfloat16
FP8 = mybir.dt.float8e4
I16 = mybir.dt.int16
I32 = mybir.dt.int32
OP = mybir.AluOpType
ACT = mybir.ActivationFunctionType

B, T, D, P, C, NEG, H = 8, 20000, 512, 128, 200, 3, 256
TT = 128      # rows per chunk (matmul contraction)
GI = 16       # chunks per indicator vector-op group
GS = 8        # chunks per image DMA slice

IMG_DT = FP8
IMG_NP = mybir.dt.np(FP8)
BF16_NP = mybir.dt.np(BF16)
DOUBLE_ROW = True    # fp8 DoubleRow matmul pairs (requires IND_DT=FP8)
IND_DT = FP8 if DOUBLE_ROW else BF16


def build_kernel(nt_pad):
    """Per-core Bass program over nt_pad compacted 128-row union chunks."""
    nc = bacc.Bacc("TRN2", target_bir_lowering=False, debug=False, num_devices=8)

    img = nc.dram_tensor("img", [TT, nt_pad * D], IMG_DT, kind="ExternalInput")
    # cf1M [128, GI*P] | len_bc [128, P]   (int16)
    meta16 = nc.dram_tensor("meta16", [TT, GI * P + P], I16, kind="ExternalInput")
    # w1 [128, 4*256] | w2 [128, 2*512] | textT [128, 4*200]
    NW1, NW2, NTX = (D // 128) * H, (H // 128) * D, (D // 128) * C
    wtx = nc.dram_tensor("wtx", [128, NW1 + NW2 + NTX], BF16, kind="ExternalInput")
    bias = nc.dram_tensor("bias", [1, H + D], BF16, kind="ExternalInput")
    # slen | lab | neg[3]
    meta32 = nc.dram_tensor("meta32", [P, 5], F32, kind="ExternalInput")
    out = nc.dram_tensor("out", [1, 1], F32, kind="ExternalOutput")

    assert nt_pad % GI == 0 and nt_pad % GS == 0
    n_gi, n_gs = nt_pad // GI, nt_pad // GS

    with tile.TileContext(nc) as tc, ExitStack() as ctx:
        con = ctx.enter_context(tc.tile_pool(name="con", bufs=1))
        ep = ctx.enter_context(tc.tile_pool(name="ep", bufs=1))
        ps_seg = ctx.enter_context(tc.tile_pool(name="ps_seg", bufs=1, space="PSUM"))
        ps_wk = ctx.enter_context(tc.tile_pool(name="ps_wk", bufs=2, space="PSUM"))
        ps_mlp = ctx.enter_context(tc.tile_pool(name="ps_mlp", bufs=1, space="PSUM"))

        # scalar-engine act-table warmup; tail order is Sqrt, Exp, Ln so end
        # the warmup on Sqrt (its reload is the only one this avoids).
        warm = con.tile([1, 1], F32)
        nc.gpsimd.memset(warm[:], 1.0)
        wo = con.tile([1, 1], F32)
        nc.scalar.activation(wo[:], warm[:], ACT.Exp)
        nc.scalar.activation(wo[:], warm[:], ACT.Ln)
        nc.scalar.activation(wo[:], warm[:], ACT.Sqrt)

        # ---- rank-offset tensors (one batched DMA) -------------------------
        m16 = con.tile([TT, GI * P + P], I16)
        nc.sync.dma_start(m16[:], meta16.ap())
        cf1M = m16[:, 0:GI * P].rearrange("t (k p) -> t k p", k=GI)
        len_ap = m16[:, GI * P:GI * P + P]
        len_bc = dataclasses.replace(
            len_ap, ap=[len_ap.ap[0], [0, GI], len_ap.ap[1]])
        cf2M = con.tile([TT, GI, P], I16)
        nc.vector.tensor_tensor(cf2M[:], cf1M, len_bc, op=OP.subtract)

        # ---- indicators: TWO fused DVE ops per GI chunks -------------------
        ind_all = con.tile([TT, nt_pad, P], IND_DT)
        gp = ctx.enter_context(tc.tile_pool(name="gp", bufs=2))
        for g in range(n_gi):
            sg = -(TT * GI) * g
            a = gp.tile([TT, GI, P], I16, tag="a")
            nc.vector.tensor_scalar(a[:], cf2M[:], sg, None, op0=OP.is_ge)
            nc.vector.scalar_tensor_tensor(
                ind_all[:, g * GI:(g + 1) * GI, :], cf1M, sg, a[:],
                op0=OP.is_ge, op1=OP.subtract)

        # ---- image stream + main indicator-matmul loop ---------------------
        img_sb = con.tile([TT, nt_pad, D], IMG_DT)
        psum_seg = ps_seg.tile([128, D], F32)
        for s in range(n_gs):
            nc.sync.dma_start(img_sb[:, s * GS:(s + 1) * GS, :],
                              img[:, s * GS * D:(s + 1) * GS * D])
            if DOUBLE_ROW:
                for j in range(GS // 2):
                    k = s * GS + 2 * j
                    nc.tensor.matmul(
                        psum_seg[:], ind_all[:, k:k + 2, :], img_sb[:, k:k + 2, :],
                        start=(k == 0), stop=(k == nt_pad - 2),
                        perf_mode=mybir.MatmulPerfMode.DoubleRow)
            else:
                for j in range(GS):
                    k = s * GS + j
                    nc.tensor.matmul(psum_seg[:], ind_all[:, k, :],
                                     img_sb[:, k, :],
                                     start=(k == 0), stop=(k == nt_pad - 1))

        # ---- deferred setup (epilogue-only inputs) -------------------------
        identity = con.tile([128, 128], BF16)
        masks.make_identity(nc, identity[:])
        ones_row = con.tile([1, 128], BF16)
        nc.gpsimd.memset(ones_row[:], 1.0)
        ones_col = con.tile([128, 1], F32)
        nc.gpsimd.memset(ones_col[:], 1.0)

        wtx_sb = con.tile([128, NW1 + NW2 + NTX], BF16)
        nc.sync.dma_start(wtx_sb[:], wtx.ap())
        w1_sb = wtx_sb[:, 0:NW1].rearrange("t (c h) -> t c h", c=D // 128)
        w2_sb = wtx_sb[:, NW1:NW1 + NW2].rearrange("t (c h) -> t c h", c=H // 128)
        textT_sb = wtx_sb[:, NW1 + NW2:].rearrange("t (c h) -> t c h", c=D // 128)
        bias_sb = con.tile([1, H + D], BF16)
        nc.sync.dma_start(bias_sb[:], bias.ap())
        b1_sb, b2_sb = bias_sb[:, 0:H], bias_sb[:, H:]

        m32 = con.tile([P, 5], F32)
        nc.sync.dma_start(m32[:], meta32.ap())
        recip_len = con.tile([P, 1], F32)
        nc.vector.reciprocal(recip_len[:], m32[:, 0:1])
        idx_bf = con.tile([P, 1 + NEG], BF16)
        nc.vector.tensor_copy(idx_bf[:], m32[:, 1:5])

        iota_c = con.tile([128, C], I32)
        nc.gpsimd.iota(iota_c[:], pattern=[[1, C]], base=0, channel_multiplier=0)
        iota_bf = con.tile([128, C], BF16)
        nc.vector.tensor_copy(iota_bf[:], iota_c[:])

        # ---- epilogue ------------------------------------------------------
        vis_b = ep.tile([128, D], BF16)
        nc.vector.tensor_scalar_mul(vis_b[:], psum_seg[:], recip_len[:])

        def transposeN(src, nblk, tag):
            dst = ep.tile([128, nblk, 128], BF16, tag=tag)
            for jd in range(nblk):
                pt = ps_wk.tile([128, 128], BF16, tag="ps_wk")
                nc.tensor.transpose(pt[:], src[:, jd * 128:(jd + 1) * 128],
                                    identity[:])
                nc.vector.tensor_copy(dst[:, jd, :], pt[:])
            return dst

        visT = transposeN(vis_b, D // 128, "visT")

        h_ps = ps_mlp.tile([128, H], F32, tag="ps_mlp")
        for c in range(D // 128):
            nc.tensor.matmul(h_ps[:], visT[:, c, :], w1_sb[:, c, :],
                             start=(c == 0), stop=False)
        nc.tensor.matmul(h_ps[:], ones_row[:], b1_sb, start=False, stop=True)
        h_sb = ep.tile([128, H], BF16)
        nc.vector.tensor_scalar_max(h_sb[:], h_ps[:], 0.0)

        hT = transposeN(h_sb, H // 128, "hT")

        o_ps = ps_mlp.tile([128, D], F32, tag="ps_o")
        for c in range(H // 128):
            nc.tensor.matmul(o_ps[:], hT[:, c, :], w2_sb[:, c, :],
                             start=(c == 0), stop=False)
        nc.tensor.matmul(o_ps[:], ones_row[:], b2_sb, start=False, stop=True)

        ov = ep.tile([128, D], BF16)
        nc.vector.tensor_tensor(ov[:], o_ps[:], vis_b[:], op=OP.add)

        # 1/||ov|| (the eps=1e-12 guard is vacuous at these magnitudes)
        sq = ep.tile([128, D], BF16)
        ssq = ep.tile([128, 1], F32)
        nc.vector.scalar_tensor_tensor(sq[:], ov[:], 0.0, ov[:], op0=OP.add,
                                       op1=OP.mult, accum_out=ssq[:])
        nrm = ep.tile([128, 1], F32)
        nc.scalar.sqrt(nrm[:], ssq[:])
        rnorm = ep.tile([128, 1], F32)
        nc.vector.reciprocal(rnorm[:], nrm[:])

        ovT = transposeN(ov, D // 128, "ovT")

        sim_ps = ps_mlp.tile([128, C], F32, tag="ps_sim")
        for c in range(D // 128):
            nc.tensor.matmul(sim_ps[:], ovT[:, c, :], textT_sb[:, c, :],
                             start=(c == 0), stop=(c == D // 128 - 1))
        # fold the 1/||ov|| normalize into the PSUM->SBUF copy of sim
        sim_sb = ep.tile([128, C], BF16)
        nc.vector.tensor_scalar_mul(sim_sb[:], sim_ps[:], rnorm[:])

        # logits[p, k] = sim[p, idx_k[p]] via one-hot masked reduction
        logits = ep.tile([128, 1 + NEG], F32)
        for k in range(1 + NEG):
            junk = ep.tile([128, C], BF16, tag="junk")
            nc.vector.scalar_tensor_tensor(
                junk[:], iota_bf[:], idx_bf[:, k:k + 1], sim_sb[:],
                op0=OP.is_equal, op1=OP.mult, accum_out=logits[:, k:k + 1])

        # loss terms: logsumexp(logits) - logits[:, 0]; |logits| <= ~6 so the
        # max-shift of the reference logsumexp is skipped (exp stays finite).
        exps = ep.tile([128, 1 + NEG], F32)
        sumexp = ep.tile([128, 1], F32)
        nc.scalar.activation(exps[:], logits[:], ACT.Exp, accum_out=sumexp[:])
        lse = ep.tile([128, 1], F32)
        nc.scalar.activation(lse[:], sumexp[:], ACT.Ln)
        term = ep.tile([128, 1], F32)
        nc.vector.tensor_tensor(term[:], lse[:], logits[:, 0:1], op=OP.subtract)

        loss_ps = ps_wk.tile([1, 1], F32, tag="ps_loss")
        nc.tensor.matmul(loss_ps[:], term[:], ones_col[:], start=True, stop=True)
        loss_sb = ep.tile([1, 1], F32)
        nc.vector.tensor_copy(loss_sb[:], loss_ps[:])
        nc.sync.dma_start(out[:], loss_sb[:])

    nc.compile()
    return nc


def prepare_shards(image_feat, seg_start, seg_len):
    """Union-row compaction. Returns per-core (img, meta16) + nt_pad."""
    nb = image_feat.shape[0]
    rows_l, rs_l, re_l = [], [], []
    for b in range(nb):
        ss = seg_start[b].astype(np.int64)
        sl = seg_len[b].astype(np.int64)
        diff = np.zeros(T + 1, np.int32)
        np.add.at(diff, ss, 1)
        np.add.at(diff, ss + sl, -1)
        rows = np.flatnonzero(np.cumsum(diff[:-1]) > 0)
        rs = np.searchsorted(rows, ss)
        re_ = np.searchsorted(rows, ss + sl)
        assert (re_ - rs == sl).all()  # segment rows are contiguous ranks
        rows_l.append(rows)
        rs_l.append(rs)
        re_l.append(re_)
    nt = max((len(r) + TT - 1) // TT for r in rows_l)
    gl = np.lcm(GI, GS)
    nt_pad = ((nt + gl - 1) // gl) * gl

    shards = []
    # cf1M[t, k', p] = t + 128*k' - rs_p ; len_bc[t, p] = seg_len_p
    tk = (np.arange(TT)[:, None] + TT * np.arange(GI)[None, :]).reshape(
        TT, GI, 1).astype(np.int64)
    for b in range(nb):
        rows = rows_l[b]
        gat = np.zeros((nt_pad * TT, D), IMG_NP)
        gat[:len(rows)] = image_feat[b][rows].astype(IMG_NP)
        img_pm = np.ascontiguousarray(
            gat.reshape(nt_pad, TT, D).transpose(1, 0, 2).reshape(TT, nt_pad * D))
        cf1M = (tk - rs_l[b].reshape(1, 1, P)).reshape(TT, GI * P)
        lenb = np.broadcast_to((re_l[b] - rs_l[b]).reshape(1, P), (TT, P))
        m16 = np.concatenate([cf1M, lenb], axis=1).astype(np.int16)
        shards.append((img_pm, np.ascontiguousarray(m16)))
    return shards, nt_pad


def make_in_maps(image_feat, text_feat, W1, b1, W2, b2, seg_start, seg_len,
                 labels, neg_idx):
    shards, nt_pad = prepare_shards(image_feat, seg_start, seg_len)
    w1r = W1.reshape(D // 128, 128, H).transpose(1, 0, 2).reshape(128, -1)
    w2r = W2.reshape(H // 128, 128, D).transpose(1, 0, 2).reshape(128, -1)
    biasr = np.concatenate([b1, b2]).reshape(1, H + D).astype(BF16_NP)
    nb = image_feat.shape[0]
    maps = []
    for c in range(nb):
        img_pm, m16 = shards[c]
        ttr = text_feat[c].T.reshape(D // 128, 128, C).transpose(1, 0, 2).reshape(
            128, -1)
        wtx = np.ascontiguousarray(
            np.concatenate([w1r, w2r, ttr], axis=1)).astype(BF16_NP)
        m32 = np.concatenate([
            seg_len[c].reshape(P, 1), labels[c].reshape(P, 1),
            neg_idx[c].reshape(P, NEG)], axis=1).astype(np.float32)
        maps.append({"img": img_pm, "meta16": m16, "wtx": wtx, "bias": biasr,
                     "meta32": m32})
    return maps, nt_pad


_NC_CACHE = {}


def _get_nc(nt_pad):
    if nt_pad not in _NC_CACHE:
        _NC_CACHE[nt_pad] = build_kernel(nt_pad)
    return _NC_CACHE[nt_pad]


def kernel(image_feat, text_feat, W1, b1, W2, b2, seg_start, seg_len, labels,
           neg_idx, _trace=False):
    in_maps, nt_pad = make_in_maps(
        np.asarray(image_feat), np.asarray(text_feat), np.asarray(W1),
        np.asarray(b1), np.asarray(W2), np.asarray(b2), np.asarray(seg_start),
        np.asarray(seg_len), np.asarray(labels), np.asarray(neg_idx))
    nc = _get_nc(nt_pad)
    nb = np.asarray(image_feat).shape[0]
    res = run_bass_kernel_spmd(nc, in_maps, core_ids=list(range(nb)), trace=_trace)
    total = sum(float(res.results[c]["out"][0, 0]) for c in range(nb))
    loss = np.float32(total / (nb * P))
    if _trace:
        return loss, res
    return loss
